# revision 1
# baseline (speedup 1.0000x reference)
"""Expert-parallel MoE MLP (BaseMLPExperts) for 8 TRN2 NeuronCores.

Reference computation (per expert e):
    y[:, e, :] = gelu_exact(x[:, e, :] @ wi[e]) @ wo[e]
with T=8192 tokens, E=8 experts, H=1024 hidden, I=4096 intermediate, fp32.

Sharding: expert-parallel — core e owns expert e (its x slice, wi[e], wo[e]).
No cross-core communication.

Per-core device kernel (all matmuls in f32r = TF32-on-PE at full PE rate,
fp32 PSUM accumulation; measured rel-err ~1.5e-4 for K=1024):
  Phase 1: h1T[I, T] = gelu(wi.T-free GEMM over xT), streamed by 512-token
           tiles; wi ([128p, 8, 4096] = 128KB/partition) SBUF-resident;
           GELU (exact erf form) applied on PSUM eviction by the ACT engine,
           written to DRAM scratch as f32r.
  Phase 2: y[T, H] = h1 @ wo, streamed by 128-token blocks; wo
           ([128p, 32, 1024] = 128KB/partition) SBUF-resident; h1T tiles act
           as the stationary matmul operand so y comes out untransposed.

Host side: transposes x slices to xT (H-major), shards, runs the SPMD kernel
on cores 0-7, stacks per-core y into [T, E, H].
"""

import numpy as np

import concourse.bass as bass
import concourse.mybir as mybir
import concourse.tile as tile
from concourse import bacc
from concourse.bass_utils import run_bass_kernel_spmd

T, E, H, I = 8192, 8, 1024, 4096
P = 128
F32 = mybir.dt.float32
F32R = mybir.dt.float32r

TT1 = 512            # phase-1 token tile
NT1 = T // TT1       # 16
HT = H // P          # 8 k-tiles for GEMM1
IT = I // P          # 32 i-tiles
TT2 = 128            # phase-2 token block
NT2 = T // TT2       # 64

# run_bass_kernel_spmd kwargs injected by test harness (e.g. trace=True)
RUN_KWARGS: dict = {}
LAST_RESULT = None

_NC = None


def _build():
    nc = bacc.Bacc("TRN2", target_bir_lowering=False, debug=False, num_devices=8)

    xT = nc.dram_tensor("xT", [H, T], F32R, kind="ExternalInput").ap()
    wi = nc.dram_tensor("wi", [H, I], F32R, kind="ExternalInput").ap()
    wo = nc.dram_tensor("wo", [I, H], F32R, kind="ExternalInput").ap()
    y = nc.dram_tensor("y", [T, H], F32, kind="ExternalOutput").ap()

    xT_r = xT.rearrange("(ho p) t -> p ho t", p=P)      # [128, 8, T]
    wi_r = wi.rearrange("(ho p) i -> p ho i", p=P)      # [128, 8, I]
    wo_r = wo.rearrange("(io p) h -> p io h", p=P)      # [128, 32, H]

    with tile.TileContext(nc) as tc:
        with tc.tile_pool(name="h1dram", bufs=1, space="DRAM") as dpool:
            # h1T scratch: one [I, TT1] block per phase-1 token tile
            h1b = [
                dpool.tile([I, TT1], F32R, name=f"h1b{t}", tag=f"h1b{t}")
                for t in range(NT1)
            ]

            # ---------------- Phase 1: h1T = gelu(x @ wi), transposed ----
            with (
                tc.tile_pool(name="wi_pool", bufs=1) as wi_pool,
                tc.tile_pool(name="xt_pool", bufs=2) as xt_pool,
                tc.tile_pool(name="h1o_pool", bufs=6) as h1o_pool,
                tc.tile_pool(name="ps1_pool", bufs=8, space="PSUM") as ps1_pool,
            ):
                wi_sb = wi_pool.tile([P, HT, I], F32R, name="wi_sb")
                for h in range(HT):
                    nc.sync.dma_start(out=wi_sb[:, h, :], in_=wi_r[:, h, :])

                for tt in range(NT1):
                    t0 = tt * TT1
                    xt = xt_pool.tile([P, HT, TT1], F32R, name="xt", tag="xt")
                    for g in range(2):
                        nc.sync.dma_start(
                            out=xt[:, 4 * g : 4 * g + 4, :],
                            in_=xT_r[:, 4 * g : 4 * g + 4, t0 : t0 + TT1],
                        )
                    for i in range(IT):
                        ps = ps1_pool.tile([P, TT1], F32, name="ps1", tag="ps1")
                        for h in range(HT):
                            nc.tensor.matmul(
                                ps[:],
                                wi_sb[:, h, i * P : (i + 1) * P],
                                xt[:, h, :],
                                start=(h == 0),
                                stop=(h == HT - 1),
                            )
                        h1o = h1o_pool.tile([P, TT1], F32R, name="h1o", tag="h1o")
                        nc.scalar.activation(
                            h1o[:], ps[:], mybir.ActivationFunctionType.Gelu
                        )
                        nc.sync.dma_start(
                            out=h1b[tt][i * P : (i + 1) * P, :], in_=h1o[:]
                        )

            # ---------------- Phase 2: y = h1 @ wo ----------------------
            with (
                tc.tile_pool(name="wo_pool", bufs=1) as wo_pool,
                tc.tile_pool(name="h1i_pool", bufs=3) as h1i_pool,
                tc.tile_pool(name="yo_pool", bufs=4) as yo_pool,
                tc.tile_pool(name="ps2_pool", bufs=8, space="PSUM") as ps2_pool,
            ):
                wo_sb = wo_pool.tile([P, IT, H], F32R, name="wo_sb")
                for g in range(4):
                    nc.sync.dma_start(
                        out=wo_sb[:, 8 * g : 8 * g + 8, :],
                        in_=wo_r[:, 8 * g : 8 * g + 8, :],
                    )

                for tb in range(NT2):
                    tt, tsub = tb // 4, tb % 4
                    src = h1b[tt].rearrange("(io p) t -> p io t", p=P)
                    h1i = h1i_pool.tile([P, IT, TT2], F32R, name="h1i", tag="h1i")
                    for g in range(4):
                        nc.sync.dma_start(
                            out=h1i[:, 8 * g : 8 * g + 8, :],
                            in_=src[
                                :, 8 * g : 8 * g + 8, tsub * TT2 : (tsub + 1) * TT2
                            ],
                        )
                    yo = yo_pool.tile([P, H], F32, name="yo", tag="yo")
                    for hh in range(2):
                        ps = ps2_pool.tile([P, 512], F32, name="ps2", tag="ps2")
                        for i in range(IT):
                            nc.tensor.matmul(
                                ps[:],
                                h1i[:, i, :],
                                wo_sb[:, i, hh * 512 : (hh + 1) * 512],
                                start=(i == 0),
                                stop=(i == IT - 1),
                            )
                        nc.vector.tensor_copy(yo[:, hh * 512 : (hh + 1) * 512], ps[:])
                    nc.sync.dma_start(
                        out=y[tb * TT2 : (tb + 1) * TT2, :], in_=yo[:]
                    )

    nc.compile()
    return nc


def kernel(x: np.ndarray, wi: np.ndarray, wo: np.ndarray) -> np.ndarray:
    global _NC, LAST_RESULT
    x = np.asarray(x, dtype=np.float32)
    wi = np.asarray(wi, dtype=np.float32)
    wo = np.asarray(wo, dtype=np.float32)
    assert x.shape == (T, E, H) and wi.shape == (E, H, I) and wo.shape == (E, I, H)

    if _NC is None:
        _NC = _build()

    in_maps = [
        {
            "xT": np.ascontiguousarray(x[:, e, :].T),
            "wi": np.ascontiguousarray(wi[e]),
            "wo": np.ascontiguousarray(wo[e]),
        }
        for e in range(E)
    ]
    res = run_bass_kernel_spmd(
        _NC, in_maps, core_ids=list(range(E)), **RUN_KWARGS
    )
    LAST_RESULT = res
    out = np.stack([res.results[e]["y"] for e in range(E)], axis=1)
    return np.ascontiguousarray(out.astype(np.float32, copy=False))


# revision 5
# speedup vs baseline: 1.0021x; 1.0021x over previous
"""Expert-parallel MoE MLP (BaseMLPExperts) for 8 TRN2 NeuronCores.

Reference computation (per expert e):
    y[:, e, :] = gelu_exact(x[:, e, :] @ wi[e]) @ wo[e]
with T=8192 tokens, E=8 experts, H=1024 hidden, I=4096 intermediate, fp32.

Sharding: expert-parallel — core e owns expert e (its x slice, wi[e], wo[e]).
No cross-core communication.

Per-core device kernel (all matmuls in f32r = TF32-on-PE at full PE rate,
fp32 PSUM accumulation; measured rel-err ~1.5e-4 for K=1024):
  Phase 1: h1T[I, T] = gelu(wi.T-free GEMM over xT), streamed by 512-token
           tiles; wi ([128p, 8, 4096] = 128KB/partition) SBUF-resident;
           GELU (exact erf form) applied on PSUM eviction by the ACT engine,
           written to DRAM scratch as f32r.
  Phase 2: y[T, H] = h1 @ wo, streamed by 128-token blocks; wo
           ([128p, 32, 1024] = 128KB/partition) SBUF-resident; h1T tiles act
           as the stationary matmul operand so y comes out untransposed.

Host side: transposes x slices to xT (H-major), shards, runs the SPMD kernel
on cores 0-7, stacks per-core y into [T, E, H].
"""

import numpy as np

import concourse.bass as bass
import concourse.mybir as mybir
import concourse.tile as tile
from concourse import bacc
from concourse.bass_utils import run_bass_kernel_spmd

T, E, H, I = 8192, 8, 1024, 4096
P = 128
F32 = mybir.dt.float32
F32R = mybir.dt.float32r

TT1 = 512            # phase-1 token tile
NT1 = T // TT1       # 16
HT = H // P          # 8 k-tiles for GEMM1
IT = I // P          # 32 i-tiles
TT2 = 128            # phase-2 token block
NT2 = T // TT2       # 64

# run_bass_kernel_spmd kwargs injected by test harness (e.g. trace=True)
RUN_KWARGS: dict = {}
LAST_RESULT = None

_NC = None


def _build():
    nc = bacc.Bacc("TRN2", target_bir_lowering=False, debug=False, num_devices=8)

    xT = nc.dram_tensor("xT", [H, T], F32R, kind="ExternalInput").ap()
    wi = nc.dram_tensor("wi", [H, I], F32R, kind="ExternalInput").ap()
    wo = nc.dram_tensor("wo", [I, H], F32R, kind="ExternalInput").ap()
    y = nc.dram_tensor("y", [T, H], F32, kind="ExternalOutput").ap()

    xT_r = xT.rearrange("(ho p) t -> p ho t", p=P)      # [128, 8, T]
    wi_r = wi.rearrange("(ho p) i -> p ho i", p=P)      # [128, 8, I]
    wo_r = wo.rearrange("(io p) h -> p io h", p=P)      # [128, 32, H]

    with tile.TileContext(nc) as tc:
        with tc.tile_pool(name="h1dram", bufs=1, space="DRAM") as dpool:
            # h1T scratch: one [I, TT1] block per phase-1 token tile
            h1b = [
                dpool.tile([I, TT1], F32R, name=f"h1b{t}", tag=f"h1b{t}")
                for t in range(NT1)
            ]

            # wo i-tiles 0..7 prefetched during phase 1 (32KB/p headroom)
            wo_pre_pool = tc.alloc_tile_pool(name="wo_pre_pool", bufs=1)
            wo_pre = wo_pre_pool.tile([P, 8, H], F32R, name="wo_pre")
            for g in range(2):
                nc.sync.dma_start(
                    out=wo_pre[:, 4 * g : 4 * g + 4, :],
                    in_=wo_r[:, 4 * g : 4 * g + 4, :],
                )

            # ---------------- Phase 1: h1T = gelu(x @ wi), transposed ----
            with (
                tc.tile_pool(name="wi_pool", bufs=1) as wi_pool,
                tc.tile_pool(name="xt_pool", bufs=2) as xt_pool,
                tc.tile_pool(name="h1o_pool", bufs=6) as h1o_pool,
                tc.tile_pool(name="ps1_pool", bufs=8, space="PSUM") as ps1_pool,
            ):
                # load wi in i-chunks ordered like phase-1 consumption, so the
                # first matmul group starts after ~2MB instead of ~17MB
                wi_sb = wi_pool.tile([P, HT, I], F32R, name="wi_sb")
                for g in range(I // 512):
                    for h in range(HT):
                        nc.sync.dma_start(
                            out=wi_sb[:, h, g * 512 : (g + 1) * 512],
                            in_=wi_r[:, h, g * 512 : (g + 1) * 512],
                        )

                for tt in range(NT1):
                    t0 = tt * TT1
                    xt = xt_pool.tile([P, HT, TT1], F32R, name="xt", tag="xt")
                    for g in range(2):
                        nc.sync.dma_start(
                            out=xt[:, 4 * g : 4 * g + 4, :],
                            in_=xT_r[:, 4 * g : 4 * g + 4, t0 : t0 + TT1],
                        )
                    for i in range(IT):
                        ps = ps1_pool.tile([P, TT1], F32, name="ps1", tag="ps1")
                        for h in range(HT):
                            nc.tensor.matmul(
                                ps[:],
                                wi_sb[:, h, i * P : (i + 1) * P],
                                xt[:, h, :],
                                start=(h == 0),
                                stop=(h == HT - 1),
                            )
                        h1o = h1o_pool.tile([P, TT1], F32R, name="h1o", tag="h1o")
                        nc.scalar.activation(
                            h1o[:], ps[:], mybir.ActivationFunctionType.Gelu
                        )
                        nc.sync.dma_start(
                            out=h1b[tt][i * P : (i + 1) * P, :], in_=h1o[:]
                        )

            # ---------------- Phase 2: y = h1 @ wo ----------------------
            with (
                tc.tile_pool(name="wo_pool", bufs=1) as wo_pool,
                tc.tile_pool(name="h1i_pool", bufs=3) as h1i_pool,
                tc.tile_pool(name="yo_pool", bufs=4) as yo_pool,
                tc.tile_pool(name="ps2_pool", bufs=8, space="PSUM") as ps2_pool,
            ):
                # i-tiles 8..31 land here as wi's SBUF frees; loaded in
                # 4-tile chunks in consumption order
                wo_sb = wo_pool.tile([P, IT - 8, H], F32R, name="wo_sb")
                for g in range(6):
                    nc.sync.dma_start(
                        out=wo_sb[:, 4 * g : 4 * g + 4, :],
                        in_=wo_r[:, 8 + 4 * g : 8 + 4 * g + 4, :],
                    )

                def wo_slice(i, hh):
                    if i < 8:
                        return wo_pre[:, i, hh * 512 : (hh + 1) * 512]
                    return wo_sb[:, i - 8, hh * 512 : (hh + 1) * 512]

                for tb in range(NT2):
                    tt, tsub = tb // 4, tb % 4
                    src = h1b[tt].rearrange("(io p) t -> p io t", p=P)
                    h1i = h1i_pool.tile([P, IT, TT2], F32R, name="h1i", tag="h1i")
                    for g in range(4):
                        nc.sync.dma_start(
                            out=h1i[:, 8 * g : 8 * g + 8, :],
                            in_=src[
                                :, 8 * g : 8 * g + 8, tsub * TT2 : (tsub + 1) * TT2
                            ],
                        )
                    yo = yo_pool.tile([P, H], F32, name="yo", tag="yo")
                    for hh in range(2):
                        ps = ps2_pool.tile([P, 512], F32, name="ps2", tag="ps2")
                        for i in range(IT):
                            nc.tensor.matmul(
                                ps[:],
                                h1i[:, i, :],
                                wo_slice(i, hh),
                                start=(i == 0),
                                stop=(i == IT - 1),
                            )
                        nc.vector.tensor_copy(yo[:, hh * 512 : (hh + 1) * 512], ps[:])
                    nc.sync.dma_start(
                        out=y[tb * TT2 : (tb + 1) * TT2, :], in_=yo[:]
                    )
            wo_pre_pool.release()

    nc.compile()
    return nc


def kernel(x: np.ndarray, wi: np.ndarray, wo: np.ndarray) -> np.ndarray:
    global _NC, LAST_RESULT
    x = np.asarray(x, dtype=np.float32)
    wi = np.asarray(wi, dtype=np.float32)
    wo = np.asarray(wo, dtype=np.float32)
    assert x.shape == (T, E, H) and wi.shape == (E, H, I) and wo.shape == (E, I, H)

    if _NC is None:
        _NC = _build()

    in_maps = [
        {
            "xT": np.ascontiguousarray(x[:, e, :].T),
            "wi": np.ascontiguousarray(wi[e]),
            "wo": np.ascontiguousarray(wo[e]),
        }
        for e in range(E)
    ]
    res = run_bass_kernel_spmd(
        _NC, in_maps, core_ids=list(range(E)), **RUN_KWARGS
    )
    LAST_RESULT = res
    out = np.stack([res.results[e]["y"] for e in range(E)], axis=1)
    return np.ascontiguousarray(out.astype(np.float32, copy=False))
